# revision 7
# baseline (speedup 1.0000x reference)
"""CustomCLIP sparse-attention kernel for 8 Trainium2 NeuronCores.

Math (per reference):
  base[b,c]  = <img_b, mt_c>
  w[b,m]     = <img_b, p_{b,m}>
  v[n,c]     = softmax_n <mt_c, t_{n,c}>
  sim[b,c,n,m] = <p_{b,m}, t_{n,c}>;  vals = top50_m(sim) sorted desc
  sel        = top50 patch indices of sim[b,0,0,:]
  w_sel[b,k] = softmax_k w[b, sel[b,k]]
  out[b,c]   = base[b,c] + sum_{k,n} vals[b,c,n,k] * w_sel[b,k] * v[n,c]

Strategy: data-parallel over batch B=32 across 8 cores (4 images/core).
Per core: stream text-feature tiles (128 (c,n)-rows, c-major), f32r PE matmul
against the core's 788 patch columns, evacuate PSUM->SBUF via ACT, then DVE
max8/match_replace rounds extract the sorted top-56 per row. The rank-weighted
sum runs as one GpSimd multiply (x * w_sel) + one DVE 3D-reduce per tile; the
v weighting and class sums happen in the finale after a DRAM restripe, in
[class, descriptor] layout, so nothing in the main loop waits on v.
"""
import os
import sys
import types
import numpy as np

B, N, ND, NC, D = 32, 197, 51, 400, 512
KTOP = 50
CORES = 8
BPC = B // CORES            # images per core
FREE = BPC * N              # 788 patch columns per core
G = NC * ND                 # 20400 (c,n) rows, c-major: g = c*51 + n
NT = (G + 127) // 128       # 160 row tiles
GP = NT * 128               # 20480 padded
K56 = 56                    # 7 rounds x 8
CBLK = 51 * 128             # 6528 contribs columns per class-block
NV = 4 * ND                 # 204 v-logit work items

LAST_EXEC_NS = None
_PROGRAM = None


def _install_ntff_hook():
    try:
        if "antenv.axon_hooks" in sys.modules:
            return
        import antenv
        mod = types.ModuleType("antenv.axon_hooks")
        _h = [None]
        mod.set_axon_ntff_profile_hook = lambda f: _h.__setitem__(0, f)
        mod.get_axon_ntff_profile_hook = lambda: _h[0]
        antenv.axon_hooks = mod
        sys.modules["antenv.axon_hooks"] = mod
        from trn_agent_boot.trn_boot import _ntff_profile_via_ctypes
        hook = _ntff_profile_via_ctypes('/opt/axon/libaxon_pjrt.so')
        if hook is not None:
            mod.set_axon_ntff_profile_hook(hook)
    except Exception:
        pass


def _build_program():
    from concourse import bacc
    import concourse.mybir as mybir
    import concourse.tile as tile

    F32 = mybir.dt.float32
    F32R = mybir.dt.float32r
    AX = mybir.AxisListType.X
    OP = mybir.AluOpType
    ACT = mybir.ActivationFunctionType

    nc = bacc.Bacc(None)

    tkc_p = nc.declare_dram_parameter("tkc", [4, 128, GP], F32R, isOutput=False)
    lkm_p = nc.declare_dram_parameter("lkm", [4, 128, FREE], F32R, isOutput=False)
    img_p = nc.declare_dram_parameter("img", [4, 128, BPC], F32R, isOutput=False)
    w5_p = nc.declare_dram_parameter("w5", [4, 128, BPC + 1], F32R, isOutput=False)
    mtk_p = nc.declare_dram_parameter("mtk", [4, 128, NC], F32R, isOutput=False)
    mtc_p = nc.declare_dram_parameter("mtc", [NC, D], F32, isOutput=False)
    acn_p = nc.declare_dram_parameter("acn", [NC, ND, D], F32, isOutput=False)
    out_p = nc.declare_dram_parameter("out", [BPC, NC], F32, isOutput=True)

    with tile.TileContext(nc) as tc:
        with tc.tile_pool(name="const", bufs=1) as cp, \
             tc.tile_pool(name="dram", bufs=1, space="DRAM") as dp, \
             tc.tile_pool(name="tk", bufs=3) as tkp, \
             tc.tile_pool(name="simp", bufs=3) as simp, \
             tc.tile_pool(name="mvp", bufs=8) as mvp, \
             tc.tile_pool(name="ctp", bufs=6) as ctp, \
             tc.tile_pool(name="scr", bufs=2) as scr, \
             tc.tile_pool(name="ps", bufs=1, space="PSUM") as pp:

            # ---------------- resident inputs ----------------
            lkm = cp.tile([128, 4, FREE], F32R)
            nc.sync.dma_start(out=lkm[:], in_=lkm_p[:].rearrange("k d f -> d k f"))
            img = cp.tile([128, 4, BPC], F32R)
            nc.sync.dma_start(out=img[:], in_=img_p[:].rearrange("k d f -> d k f"))
            w5 = cp.tile([128, 4, BPC + 1], F32R)
            nc.sync.dma_start(out=w5[:], in_=w5_p[:].rearrange("k d f -> d k f"))
            mtk = cp.tile([128, 4, NC], F32R)
            nc.sync.dma_start(out=mtk[:], in_=mtk_p[:].rearrange("k d f -> d k f"))

            contribs_d = dp.tile([BPC, GP], F32)

            # ---------------- phase W: w_sel -----------------
            ps_w = pp.tile([BPC + 1, FREE], F32, bufs=1)
            for k in range(4):
                nc.tensor.matmul(ps_w[:, 0:512], w5[:, k, :], lkm[:, k, 0:512],
                                 start=(k == 0), stop=(k == 3))
                nc.tensor.matmul(ps_w[:, 512:FREE], w5[:, k, :], lkm[:, k, 512:FREE],
                                 start=(k == 0), stop=(k == 3))
            ws_all = cp.tile([BPC + 1, FREE], F32)
            nc.scalar.copy(out=ws_all[:], in_=ps_w[:])

            s04 = cp.tile([BPC, N], F32)
            nc.sync.dma_start(out=s04[:], in_=ws_all[BPC:BPC + 1, :])
            w4 = cp.tile([BPC, N], F32)
            for b in range(BPC):
                nc.sync.dma_start(out=w4[b:b + 1, :],
                                  in_=ws_all[b:b + 1, b * N:(b + 1) * N])

            s0keep = cp.tile([BPC, N], F32)
            nc.scalar.copy(out=s0keep[:], in_=s04[:])
            m56 = cp.tile([BPC, K56], F32)
            for r in range(7):
                nc.vector.max(out=m56[:, r * 8:(r + 1) * 8], in_=s04[:])
                if r < 6:
                    nc.vector.match_replace(out=s04[:],
                                            in_to_replace=m56[:, r * 8:(r + 1) * 8],
                                            in_values=s04[:], imm_value=-1e30)

            # gather w at the top-k positions: onehot[k,m] = (s0[m] == m56[k])
            eq3 = cp.tile([BPC, K56 * N], F32)
            w4b = w4[:].rearrange("p (o m) -> p o m", o=1).to_broadcast([BPC, K56, N])
            s0b = s0keep[:].rearrange("p (o m) -> p o m", o=1).to_broadcast([BPC, K56, N])
            m56b = m56[:].rearrange("p (k o) -> p k o", o=1).to_broadcast([BPC, K56, N])
            nc.vector.tensor_tensor(out=eq3[:].rearrange("p (a m) -> p a m", a=K56),
                                    in0=m56b, in1=s0b, op=OP.is_equal)
            nc.vector.tensor_tensor(out=eq3[:].rearrange("p (a m) -> p a m", a=K56),
                                    in0=eq3[:].rearrange("p (a m) -> p a m", a=K56),
                                    in1=w4b, op=OP.mult)
            wg = cp.tile([BPC, K56], F32)
            nc.vector.reduce_sum(out=wg[:], in_=eq3[:].rearrange("p (a m) -> p a m", a=K56),
                                 axis=AX)

            wselp = cp.tile([BPC, K56], F32)
            nc.vector.memset(wselp[:], 0.0)
            wsum = cp.tile([BPC, 1], F32)
            nc.scalar.activation(out=wselp[:, 0:KTOP], in_=wg[:, 0:KTOP],
                                 func=ACT.Exp, accum_out=wsum[:])
            wrec = cp.tile([BPC, 1], F32)
            nc.vector.reciprocal(out=wrec[:], in_=wsum[:])
            nc.vector.tensor_scalar_mul(wselp[:, 0:KTOP], wselp[:, 0:KTOP], wrec[:])

            wflat = cp.tile([1, BPC * K56], F32)
            nc.sync.dma_start(out=wflat[:], in_=wselp[:])
            ones = cp.tile([1, 128], F32)
            nc.vector.memset(ones[:], 1.0)
            bc_ps = pp.tile([128, BPC * K56], F32, bufs=1)
            nc.tensor.matmul(bc_ps[:], ones[:], wflat[:], start=True, stop=True)
            wrep = cp.tile([128, BPC * K56], F32)
            nc.scalar.copy(out=wrep[:], in_=bc_ps[:])

            # ------------- phase V state (filled inside main loop) -------------
            mtcbs, vlogs, vexps = [], [], []
            for cb in range(4):
                mtcbs.append(cp.tile([128, D], F32, tag=f"mtc{cb}", name=f"mtcb{cb}"))
                vlogs.append(cp.tile([128, ND], F32, tag=f"vlog{cb}", name=f"vlog{cb}"))
                vexps.append(cp.tile([128, ND], F32, tag=f"vexp{cb}", name=f"vexp{cb}"))
            for cb in range(4):
                cr = min(128, NC - cb * 128)
                nc.sync.dma_start(out=mtcbs[cb][:cr, :],
                                  in_=mtc_p[cb * 128:cb * 128 + cr, :])

            def v_item(j):
                cb, n = j // ND, j % ND
                cr = min(128, NC - cb * 128)
                acn_t = scr.tile([128, D], F32, tag="acn", bufs=3, name=f"acn{j}")
                nc.gpsimd.dma_start(out=acn_t[:cr, :],
                                    in_=acn_p[cb * 128:cb * 128 + cr, n, :])
                vj = scr.tile([128, D], F32, tag="vjunk", bufs=3, name=f"vj{j}")
                nc.gpsimd.tensor_tensor(out=vj[:cr, :], in0=acn_t[:cr, :],
                                        in1=mtcbs[cb][:cr, :], op=OP.mult)
                vj2 = scr.tile([128, D], F32, tag="vjunk2", bufs=2, name=f"vj2{j}")
                nc.scalar.activation(out=vj2[:cr, :], in_=vj[:cr, :],
                                     func=ACT.Copy,
                                     accum_out=vlogs[cb][:cr, n:n + 1])
                if n == ND - 1:
                    vsum = cp.tile([128, 1], F32, tag=f"vsum{cb}", name=f"vsum{cb}")
                    nc.scalar.activation(out=vexps[cb][:cr, :], in_=vlogs[cb][:cr, :],
                                         func=ACT.Exp, accum_out=vsum[:cr, :])
                    vrec = cp.tile([128, 1], F32, tag=f"vrec{cb}", name=f"vrec{cb}")
                    nc.vector.reciprocal(out=vrec[:cr, :], in_=vsum[:cr, :])
                    nc.vector.tensor_scalar_mul(vexps[cb][:cr, :], vexps[cb][:cr, :],
                                                vrec[:cr, :])

            # ---------------- main loop ----------------------
            for t in range(NT):
                tkt = tkp.tile([128, 4, 128], F32R)
                nc.sync.dma_start(out=tkt[:],
                                  in_=tkc_p[:, :, t * 128:(t + 1) * 128]
                                  .rearrange("k d f -> d k f"))
                st = pp.tile([128, FREE], F32, tag="st", bufs=2)
                for k in range(4):
                    nc.tensor.matmul(st[:, 0:512], tkt[:, k, :], lkm[:, k, 0:512],
                                     start=(k == 0), stop=(k == 3))
                    nc.tensor.matmul(st[:, 512:FREE], tkt[:, k, :], lkm[:, k, 512:FREE],
                                     start=(k == 0), stop=(k == 3))
                sim = simp.tile([128, FREE], F32, tag="sim")
                nc.scalar.copy(out=sim[:], in_=st[:])

                mv3 = mvp.tile([128, BPC, K56], F32, tag="maxv", name=f"mv_{t}")
                for r in range(7):
                    for b in range(BPC):
                        nc.vector.max(out=mv3[:, b, r * 8:(r + 1) * 8],
                                      in_=sim[:, b * N:(b + 1) * N])
                    if r < 6:
                        for b in range(BPC):
                            nc.vector.match_replace(
                                out=sim[:, b * N:(b + 1) * N],
                                in_to_replace=mv3[:, b, r * 8:(r + 1) * 8],
                                in_values=sim[:, b * N:(b + 1) * N],
                                imm_value=-1e30)

                prod = scr.tile([128, BPC * K56], F32, tag="prod", bufs=3,
                                name=f"prod{t}")
                nc.gpsimd.tensor_tensor(out=prod[:],
                                        in0=mv3[:].rearrange("p a k -> p (a k)"),
                                        in1=wrep[:], op=OP.mult)
                ct = ctp.tile([128, BPC], F32, tag="ct", name=f"ct{t}")
                nc.vector.reduce_sum(out=ct[:],
                                     in_=prod[:].rearrange("p (a k) -> p a k", a=BPC),
                                     axis=AX)
                nc.sync.dma_start(
                    out=contribs_d[:, t * 128:(t + 1) * 128].rearrange("b p -> p b"),
                    in_=ct[:])

                # interleave v work across the first tiles
                for j in range(t * NV // NT, (t + 1) * NV // NT):
                    v_item(j)

            # ---------------- finale -------------------------
            for cb in range(4):
                cr = min(128, NC - cb * 128)
                rb = cp.tile([128, BPC * ND], F32, tag=f"rb{cb}", name=f"rb{cb}")
                nc.sync.dma_start(
                    out=rb[:cr, :],
                    in_=contribs_d[:, cb * CBLK:cb * CBLK + cr * ND]
                    .rearrange("b (p n) -> p b n", n=ND))
                vb = vexps[cb][:cr, :].rearrange("p (o n) -> p o n", o=1) \
                    .to_broadcast([cr, BPC, ND])
                nc.vector.tensor_tensor(out=rb[:cr, :].rearrange("p (b n) -> p b n", n=ND),
                                        in0=rb[:cr, :].rearrange("p (b n) -> p b n", n=ND),
                                        in1=vb, op=OP.mult)
                bias4 = cp.tile([128, BPC], F32, tag=f"bias{cb}", name=f"bias{cb}")
                nc.vector.reduce_sum(out=bias4[:cr, :],
                                     in_=rb[:cr, :].rearrange("p (b n) -> p b n", n=ND),
                                     axis=AX)
                pb = pp.tile([128, BPC], F32, tag="pb", bufs=1)
                for k in range(4):
                    nc.tensor.matmul(pb[:cr, :], mtk[:, k, cb * 128:cb * 128 + cr],
                                     img[:, k, :], start=(k == 0), stop=(k == 3))
                o4 = cp.tile([128, BPC], F32, tag=f"o4{cb}", name=f"o4{cb}")
                nc.vector.tensor_tensor(out=o4[:cr, :], in0=bias4[:cr, :],
                                        in1=pb[:cr, :], op=OP.add)
                nc.sync.dma_start(
                    out=out_p[:, cb * 128:cb * 128 + cr].rearrange("b c -> c b"),
                    in_=o4[:cr, :])

    nc.finalize()
    return nc


def kernel(image_features, local_image_features, all_text_features,
           mean_text_features, topk):
    global LAST_EXEC_NS, _PROGRAM
    assert int(topk) == KTOP
    _install_ntff_hook()
    from concourse.bass_utils import run_bass_kernel_spmd

    imgf = np.ascontiguousarray(np.asarray(image_features, dtype=np.float32))
    locf = np.ascontiguousarray(np.asarray(local_image_features, dtype=np.float32))
    txtf = np.ascontiguousarray(np.asarray(all_text_features, dtype=np.float32))
    mtf = np.ascontiguousarray(np.asarray(mean_text_features, dtype=np.float32))

    # text cols c-major: col j = c*51+n  ->  all_text[n,c,:]
    tkc = np.zeros((4, 128, GP), dtype=np.float32)
    tkc[:, :, :G] = txtf.transpose(2, 1, 0).reshape(D, G).reshape(4, 128, G)
    mtk = mtf.T.reshape(4, 128, NC).copy()
    acn = txtf.transpose(1, 0, 2).copy()           # [c, n, d]
    t00 = txtf[0, 0, :]                            # class 0, descriptor 0

    if _PROGRAM is None:
        _PROGRAM = _build_program()
    nc = _PROGRAM

    in_maps = []
    for ci in range(CORES):
        sl = slice(ci * BPC, (ci + 1) * BPC)
        li = locf[sl]                              # [4, 197, 512]
        lkm = li.transpose(2, 0, 1).reshape(D, FREE).reshape(4, 128, FREE).copy()
        im = imgf[sl].T.reshape(4, 128, BPC).copy()
        w5 = np.concatenate([imgf[sl].T, t00[:, None]], axis=1) \
            .reshape(4, 128, BPC + 1).copy()
        in_maps.append({
            "tkc": tkc, "lkm": lkm, "img": im, "w5": w5,
            "mtk": mtk, "mtc": mtf, "acn": acn,
        })

    res = run_bass_kernel_spmd(nc, in_maps, core_ids=list(range(CORES)))
    LAST_EXEC_NS = res.exec_time_ns
    out = np.concatenate([res.results[ci]["out"] for ci in range(CORES)], axis=0)
    return out.astype(np.float32)


# revision 9
# speedup vs baseline: 1.1615x; 1.1615x over previous
"""CustomCLIP sparse-attention kernel for 8 Trainium2 NeuronCores.

Math (per reference):
  base[b,c]  = <img_b, mt_c>
  w[b,m]     = <img_b, p_{b,m}>
  v[n,c]     = softmax_n <mt_c, t_{n,c}>
  sim[b,c,n,m] = <p_{b,m}, t_{n,c}>;  vals = top50_m(sim) sorted desc
  sel        = top50 patch indices of sim[b,0,0,:]
  w_sel[b,k] = softmax_k w[b, sel[b,k]]
  out[b,c]   = base[b,c] + sum_{k,n} vals[b,c,n,k] * w_sel[b,k] * v[n,c]

Strategy: data-parallel over batch B=32 across 8 cores (4 images/core).
Per core: stream text-feature tiles (128 (c,n)-rows, c-major), f32r PE matmul
against the core's 788 patch columns, evacuate PSUM->SBUF via ACT, then DVE
max8/match_replace rounds extract the sorted top-56 per row. The rank-weighted
sum runs as one GpSimd multiply (x * w_sel) + one DVE 3D-reduce per tile; the
v weighting and class sums happen in the finale after a DRAM restripe, in
[class, descriptor] layout, so nothing in the main loop waits on v.
"""
import os
import sys
import types
import numpy as np

B, N, ND, NC, D = 32, 197, 51, 400, 512
KTOP = 50
CORES = 8
BPC = B // CORES            # images per core
FREE = BPC * N              # 788 patch columns per core
G = NC * ND                 # 20400 (c,n) rows, c-major: g = c*51 + n
NT = (G + 127) // 128       # 160 row tiles
GP = NT * 128               # 20480 padded
K56 = 56                    # 7 rounds x 8
CBLK = 51 * 128             # 6528 contribs columns per class-block
NV = 4 * ND                 # 204 v-logit work items

LAST_EXEC_NS = None
_PROGRAM = None


def _install_ntff_hook():
    try:
        if "antenv.axon_hooks" in sys.modules:
            return
        import antenv
        mod = types.ModuleType("antenv.axon_hooks")
        _h = [None]
        mod.set_axon_ntff_profile_hook = lambda f: _h.__setitem__(0, f)
        mod.get_axon_ntff_profile_hook = lambda: _h[0]
        antenv.axon_hooks = mod
        sys.modules["antenv.axon_hooks"] = mod
        from trn_agent_boot.trn_boot import _ntff_profile_via_ctypes
        hook = _ntff_profile_via_ctypes('/opt/axon/libaxon_pjrt.so')
        if hook is not None:
            mod.set_axon_ntff_profile_hook(hook)
    except Exception:
        pass


def _build_program():
    from concourse import bacc
    import concourse.mybir as mybir
    import concourse.tile as tile

    F32 = mybir.dt.float32
    F32R = mybir.dt.float32r
    AX = mybir.AxisListType.X
    OP = mybir.AluOpType
    ACT = mybir.ActivationFunctionType

    nc = bacc.Bacc(None)

    tkc_p = nc.declare_dram_parameter("tkc", [4, 128, GP], F32R, isOutput=False)
    lkm_p = nc.declare_dram_parameter("lkm", [4, 128, FREE], F32R, isOutput=False)
    img_p = nc.declare_dram_parameter("img", [4, 128, BPC], F32R, isOutput=False)
    w5_p = nc.declare_dram_parameter("w5", [4, 128, BPC + 1], F32R, isOutput=False)
    mtk_p = nc.declare_dram_parameter("mtk", [4, 128, NC], F32R, isOutput=False)
    mtc_p = nc.declare_dram_parameter("mtc", [NC, D], F32, isOutput=False)
    acn_p = nc.declare_dram_parameter("acn", [NC, ND, D], F32, isOutput=False)
    out_p = nc.declare_dram_parameter("out", [BPC, NC], F32, isOutput=True)

    with tile.TileContext(nc) as tc:
        with tc.tile_pool(name="const", bufs=1) as cp, \
             tc.tile_pool(name="dram", bufs=1, space="DRAM") as dp, \
             tc.tile_pool(name="tk", bufs=3) as tkp, \
             tc.tile_pool(name="simp", bufs=3) as simp, \
             tc.tile_pool(name="mvp", bufs=8) as mvp, \
             tc.tile_pool(name="ctp", bufs=6) as ctp, \
             tc.tile_pool(name="scr", bufs=2) as scr, \
             tc.tile_pool(name="ps", bufs=1, space="PSUM") as pp:

            # ---------------- resident inputs ----------------
            lkm = cp.tile([128, 4, FREE], F32R)
            nc.sync.dma_start(out=lkm[:], in_=lkm_p[:].rearrange("k d f -> d k f"))
            img = cp.tile([128, 4, BPC], F32R)
            nc.sync.dma_start(out=img[:], in_=img_p[:].rearrange("k d f -> d k f"))
            w5 = cp.tile([128, 4, BPC + 1], F32R)
            nc.sync.dma_start(out=w5[:], in_=w5_p[:].rearrange("k d f -> d k f"))
            mtk = cp.tile([128, 4, NC], F32R)
            nc.sync.dma_start(out=mtk[:], in_=mtk_p[:].rearrange("k d f -> d k f"))

            contribs_d = dp.tile([BPC, GP], F32)

            # ---------------- phase W: w_sel -----------------
            ps_w = pp.tile([BPC + 1, FREE], F32, bufs=1)
            for k in range(4):
                nc.tensor.matmul(ps_w[:, 0:512], w5[:, k, :], lkm[:, k, 0:512],
                                 start=(k == 0), stop=(k == 3))
                nc.tensor.matmul(ps_w[:, 512:FREE], w5[:, k, :], lkm[:, k, 512:FREE],
                                 start=(k == 0), stop=(k == 3))
            ws_all = cp.tile([BPC + 1, FREE], F32)
            nc.scalar.copy(out=ws_all[:], in_=ps_w[:])

            s04 = cp.tile([BPC, N], F32)
            nc.sync.dma_start(out=s04[:], in_=ws_all[BPC:BPC + 1, :])
            w4 = cp.tile([BPC, N], F32)
            for b in range(BPC):
                nc.sync.dma_start(out=w4[b:b + 1, :],
                                  in_=ws_all[b:b + 1, b * N:(b + 1) * N])

            s0keep = cp.tile([BPC, N], F32)
            nc.scalar.copy(out=s0keep[:], in_=s04[:])
            m56 = cp.tile([BPC, K56], F32)
            for r in range(7):
                nc.vector.max(out=m56[:, r * 8:(r + 1) * 8], in_=s04[:])
                if r < 6:
                    nc.vector.match_replace(out=s04[:],
                                            in_to_replace=m56[:, r * 8:(r + 1) * 8],
                                            in_values=s04[:], imm_value=-1e30)

            # gather w at the top-k positions: onehot[k,m] = (s0[m] == m56[k])
            eq3 = cp.tile([BPC, K56 * N], F32)
            w4b = w4[:].rearrange("p (o m) -> p o m", o=1).to_broadcast([BPC, K56, N])
            s0b = s0keep[:].rearrange("p (o m) -> p o m", o=1).to_broadcast([BPC, K56, N])
            m56b = m56[:].rearrange("p (k o) -> p k o", o=1).to_broadcast([BPC, K56, N])
            nc.vector.tensor_tensor(out=eq3[:].rearrange("p (a m) -> p a m", a=K56),
                                    in0=m56b, in1=s0b, op=OP.is_equal)
            nc.vector.tensor_tensor(out=eq3[:].rearrange("p (a m) -> p a m", a=K56),
                                    in0=eq3[:].rearrange("p (a m) -> p a m", a=K56),
                                    in1=w4b, op=OP.mult)
            wg = cp.tile([BPC, K56], F32)
            nc.vector.reduce_sum(out=wg[:], in_=eq3[:].rearrange("p (a m) -> p a m", a=K56),
                                 axis=AX)

            wselp = cp.tile([BPC, K56], F32)
            nc.vector.memset(wselp[:], 0.0)
            wsum = cp.tile([BPC, 1], F32)
            nc.scalar.activation(out=wselp[:, 0:KTOP], in_=wg[:, 0:KTOP],
                                 func=ACT.Exp, accum_out=wsum[:])
            wrec = cp.tile([BPC, 1], F32)
            nc.vector.reciprocal(out=wrec[:], in_=wsum[:])
            nc.vector.tensor_scalar_mul(wselp[:, 0:KTOP], wselp[:, 0:KTOP], wrec[:])

            wflat = cp.tile([1, BPC * K56], F32)
            nc.sync.dma_start(out=wflat[:], in_=wselp[:])
            ones = cp.tile([1, 128], F32)
            nc.vector.memset(ones[:], 1.0)
            bc_ps = pp.tile([128, BPC * K56], F32, bufs=1)
            nc.tensor.matmul(bc_ps[:], ones[:], wflat[:], start=True, stop=True)
            wrep = cp.tile([128, BPC * K56], F32)
            nc.scalar.copy(out=wrep[:], in_=bc_ps[:])

            # ------------- phase V state (filled inside main loop) -------------
            mtcbs, vlogs, vexps = [], [], []
            for cb in range(4):
                mtcbs.append(cp.tile([128, D], F32, tag=f"mtc{cb}", name=f"mtcb{cb}"))
                vlogs.append(cp.tile([128, ND], F32, tag=f"vlog{cb}", name=f"vlog{cb}"))
                vexps.append(cp.tile([128, ND], F32, tag=f"vexp{cb}", name=f"vexp{cb}"))
            for cb in range(4):
                cr = min(128, NC - cb * 128)
                nc.sync.dma_start(out=mtcbs[cb][:cr, :],
                                  in_=mtc_p[cb * 128:cb * 128 + cr, :])

            def v_item(j):
                cb, n = j // ND, j % ND
                cr = min(128, NC - cb * 128)
                acn_t = scr.tile([128, D], F32, tag="acn", bufs=3, name=f"acn{j}")
                nc.gpsimd.dma_start(out=acn_t[:cr, :],
                                    in_=acn_p[cb * 128:cb * 128 + cr, n, :])
                vj = scr.tile([128, D], F32, tag="vjunk", bufs=3, name=f"vj{j}")
                nc.gpsimd.tensor_tensor(out=vj[:cr, :], in0=acn_t[:cr, :],
                                        in1=mtcbs[cb][:cr, :], op=OP.mult)
                vj2 = scr.tile([128, D], F32, tag="vjunk2", bufs=2, name=f"vj2{j}")
                nc.scalar.activation(out=vj2[:cr, :], in_=vj[:cr, :],
                                     func=ACT.Copy,
                                     accum_out=vlogs[cb][:cr, n:n + 1])
                if n == ND - 1:
                    vsum = cp.tile([128, 1], F32, tag=f"vsum{cb}", name=f"vsum{cb}")
                    nc.scalar.activation(out=vexps[cb][:cr, :], in_=vlogs[cb][:cr, :],
                                         func=ACT.Exp, accum_out=vsum[:cr, :])
                    vrec = cp.tile([128, 1], F32, tag=f"vrec{cb}", name=f"vrec{cb}")
                    nc.vector.reciprocal(out=vrec[:cr, :], in_=vsum[:cr, :])
                    nc.vector.tensor_scalar_mul(vexps[cb][:cr, :], vexps[cb][:cr, :],
                                                vrec[:cr, :])

            # ---------------- main loop ----------------------
            LAG = 3
            pending = []

            def flush_tail(t):
                tt, mv = pending.pop(0)
                prod = scr.tile([128, BPC * K56], F32, tag="prod", bufs=3,
                                name=f"prod{tt}")
                nc.gpsimd.tensor_tensor(out=prod[:],
                                        in0=mv[:].rearrange("p a k -> p (a k)"),
                                        in1=wrep[:], op=OP.mult)
                ct = ctp.tile([128, BPC], F32, tag="ct", name=f"ct{tt}")
                nc.vector.reduce_sum(out=ct[:],
                                     in_=prod[:].rearrange("p (a k) -> p a k", a=BPC),
                                     axis=AX)
                nc.sync.dma_start(
                    out=contribs_d[:, tt * 128:(tt + 1) * 128].rearrange("b p -> p b"),
                    in_=ct[:])

            for t in range(NT):
                tkt = tkp.tile([128, 4, 128], F32R)
                nc.sync.dma_start(out=tkt[:],
                                  in_=tkc_p[:, :, t * 128:(t + 1) * 128]
                                  .rearrange("k d f -> d k f"))
                st = pp.tile([128, FREE], F32, tag="st", bufs=2)
                for k in range(4):
                    nc.tensor.matmul(st[:, 0:512], tkt[:, k, :], lkm[:, k, 0:512],
                                     start=(k == 0), stop=(k == 3))
                    nc.tensor.matmul(st[:, 512:FREE], tkt[:, k, :], lkm[:, k, 512:FREE],
                                     start=(k == 0), stop=(k == 3))
                sim = simp.tile([128, FREE], F32, tag="sim")
                nc.scalar.copy(out=sim[:], in_=st[:])

                mv3 = mvp.tile([128, BPC, K56], F32, tag="maxv", name=f"mv_{t}")
                for r in range(7):
                    for b in range(BPC):
                        nc.vector.max(out=mv3[:, b, r * 8:(r + 1) * 8],
                                      in_=sim[:, b * N:(b + 1) * N])
                    if r < 6:
                        for b in range(BPC):
                            nc.vector.match_replace(
                                out=sim[:, b * N:(b + 1) * N],
                                in_to_replace=mv3[:, b, r * 8:(r + 1) * 8],
                                in_values=sim[:, b * N:(b + 1) * N],
                                imm_value=-1e30)

                pending.append((t, mv3))
                if len(pending) > LAG:
                    flush_tail(t)

                # interleave v work across the first tiles
                for j in range(t * NV // NT, (t + 1) * NV // NT):
                    v_item(j)

            while pending:
                flush_tail(NT)

            # ---------------- finale -------------------------
            for cb in range(4):
                cr = min(128, NC - cb * 128)
                rb = cp.tile([128, BPC * ND], F32, tag=f"rb{cb}", name=f"rb{cb}")
                nc.sync.dma_start(
                    out=rb[:cr, :],
                    in_=contribs_d[:, cb * CBLK:cb * CBLK + cr * ND]
                    .rearrange("b (p n) -> p b n", n=ND))
                vb = vexps[cb][:cr, :].rearrange("p (o n) -> p o n", o=1) \
                    .to_broadcast([cr, BPC, ND])
                nc.vector.tensor_tensor(out=rb[:cr, :].rearrange("p (b n) -> p b n", n=ND),
                                        in0=rb[:cr, :].rearrange("p (b n) -> p b n", n=ND),
                                        in1=vb, op=OP.mult)
                bias4 = cp.tile([128, BPC], F32, tag=f"bias{cb}", name=f"bias{cb}")
                nc.vector.reduce_sum(out=bias4[:cr, :],
                                     in_=rb[:cr, :].rearrange("p (b n) -> p b n", n=ND),
                                     axis=AX)
                pb = pp.tile([128, BPC], F32, tag="pb", bufs=1)
                for k in range(4):
                    nc.tensor.matmul(pb[:cr, :], mtk[:, k, cb * 128:cb * 128 + cr],
                                     img[:, k, :], start=(k == 0), stop=(k == 3))
                o4 = cp.tile([128, BPC], F32, tag=f"o4{cb}", name=f"o4{cb}")
                nc.vector.tensor_tensor(out=o4[:cr, :], in0=bias4[:cr, :],
                                        in1=pb[:cr, :], op=OP.add)
                nc.sync.dma_start(
                    out=out_p[:, cb * 128:cb * 128 + cr].rearrange("b c -> c b"),
                    in_=o4[:cr, :])

    nc.finalize()
    return nc


def kernel(image_features, local_image_features, all_text_features,
           mean_text_features, topk):
    global LAST_EXEC_NS, _PROGRAM
    assert int(topk) == KTOP
    _install_ntff_hook()
    from concourse.bass_utils import run_bass_kernel_spmd

    imgf = np.ascontiguousarray(np.asarray(image_features, dtype=np.float32))
    locf = np.ascontiguousarray(np.asarray(local_image_features, dtype=np.float32))
    txtf = np.ascontiguousarray(np.asarray(all_text_features, dtype=np.float32))
    mtf = np.ascontiguousarray(np.asarray(mean_text_features, dtype=np.float32))

    # text cols c-major: col j = c*51+n  ->  all_text[n,c,:]
    tkc = np.zeros((4, 128, GP), dtype=np.float32)
    tkc[:, :, :G] = txtf.transpose(2, 1, 0).reshape(D, G).reshape(4, 128, G)
    mtk = mtf.T.reshape(4, 128, NC).copy()
    acn = txtf.transpose(1, 0, 2).copy()           # [c, n, d]
    t00 = txtf[0, 0, :]                            # class 0, descriptor 0

    if _PROGRAM is None:
        _PROGRAM = _build_program()
    nc = _PROGRAM

    in_maps = []
    for ci in range(CORES):
        sl = slice(ci * BPC, (ci + 1) * BPC)
        li = locf[sl]                              # [4, 197, 512]
        lkm = li.transpose(2, 0, 1).reshape(D, FREE).reshape(4, 128, FREE).copy()
        im = imgf[sl].T.reshape(4, 128, BPC).copy()
        w5 = np.concatenate([imgf[sl].T, t00[:, None]], axis=1) \
            .reshape(4, 128, BPC + 1).copy()
        in_maps.append({
            "tkc": tkc, "lkm": lkm, "img": im, "w5": w5,
            "mtk": mtk, "mtc": mtf, "acn": acn,
        })

    res = run_bass_kernel_spmd(nc, in_maps, core_ids=list(range(CORES)))
    LAST_EXEC_NS = res.exec_time_ns
    out = np.concatenate([res.results[ci]["out"] for ci in range(CORES)], axis=0)
    return out.astype(np.float32)


# revision 10
# speedup vs baseline: 1.1956x; 1.0294x over previous
"""CustomCLIP sparse-attention kernel for 8 Trainium2 NeuronCores.

Math (per reference):
  base[b,c]  = <img_b, mt_c>
  w[b,m]     = <img_b, p_{b,m}>
  v[n,c]     = softmax_n <mt_c, t_{n,c}>
  sim[b,c,n,m] = <p_{b,m}, t_{n,c}>;  vals = top50_m(sim) sorted desc
  sel        = top50 patch indices of sim[b,0,0,:]
  w_sel[b,k] = softmax_k w[b, sel[b,k]]
  out[b,c]   = base[b,c] + sum_{k,n} vals[b,c,n,k] * w_sel[b,k] * v[n,c]

Strategy: data-parallel over batch B=32 across 8 cores (4 images/core).
Per core: stream text-feature tiles (128 (c,n)-rows, c-major), f32r PE matmul
against the core's 788 patch columns, evacuate PSUM->SBUF via ACT, then DVE
max8/match_replace rounds extract the sorted top-56 per row. The rank-weighted
sum runs as one GpSimd multiply (x * w_sel) + one DVE 3D-reduce per tile; the
v weighting and class sums happen in the finale after a DRAM restripe, in
[class, descriptor] layout, so nothing in the main loop waits on v.
"""
import os
import sys
import types
import numpy as np

B, N, ND, NC, D = 32, 197, 51, 400, 512
KTOP = 50
CORES = 8
BPC = B // CORES            # images per core
FREE = BPC * N              # 788 patch columns per core
G = NC * ND                 # 20400 (c,n) rows, c-major: g = c*51 + n
NT = (G + 127) // 128       # 160 row tiles
GP = NT * 128               # 20480 padded
K56 = 56                    # 7 rounds x 8
CBLK = 51 * 128             # 6528 contribs columns per class-block
NV = 4 * ND                 # 204 v-logit work items

LAST_EXEC_NS = None
_PROGRAM = None


def _install_ntff_hook():
    try:
        if "antenv.axon_hooks" in sys.modules:
            return
        import antenv
        mod = types.ModuleType("antenv.axon_hooks")
        _h = [None]
        mod.set_axon_ntff_profile_hook = lambda f: _h.__setitem__(0, f)
        mod.get_axon_ntff_profile_hook = lambda: _h[0]
        antenv.axon_hooks = mod
        sys.modules["antenv.axon_hooks"] = mod
        from trn_agent_boot.trn_boot import _ntff_profile_via_ctypes
        hook = _ntff_profile_via_ctypes('/opt/axon/libaxon_pjrt.so')
        if hook is not None:
            mod.set_axon_ntff_profile_hook(hook)
    except Exception:
        pass


def _build_program():
    from concourse import bacc
    import concourse.mybir as mybir
    import concourse.tile as tile

    F32 = mybir.dt.float32
    F32R = mybir.dt.float32r
    AX = mybir.AxisListType.X
    OP = mybir.AluOpType
    ACT = mybir.ActivationFunctionType

    nc = bacc.Bacc(None)

    tkc_p = nc.declare_dram_parameter("tkc", [NT, 128, 512], F32R, isOutput=False)
    lkm_p = nc.declare_dram_parameter("lkm", [4, 128, FREE], F32R, isOutput=False)
    img_p = nc.declare_dram_parameter("img", [4, 128, BPC], F32R, isOutput=False)
    w5_p = nc.declare_dram_parameter("w5", [4, 128, BPC + 1], F32R, isOutput=False)
    mtk_p = nc.declare_dram_parameter("mtk", [4, 128, NC], F32R, isOutput=False)
    mtc_p = nc.declare_dram_parameter("mtc", [NC, D], F32, isOutput=False)
    acn_p = nc.declare_dram_parameter("acn", [NC, ND, D], F32, isOutput=False)
    out_p = nc.declare_dram_parameter("out", [BPC, NC], F32, isOutput=True)

    with tile.TileContext(nc) as tc:
        with tc.tile_pool(name="const", bufs=1) as cp, \
             tc.tile_pool(name="dram", bufs=1, space="DRAM") as dp, \
             tc.tile_pool(name="tk", bufs=3) as tkp, \
             tc.tile_pool(name="simp", bufs=3) as simp, \
             tc.tile_pool(name="mvp", bufs=8) as mvp, \
             tc.tile_pool(name="ctp", bufs=6) as ctp, \
             tc.tile_pool(name="scr", bufs=2) as scr, \
             tc.tile_pool(name="ps", bufs=1, space="PSUM") as pp:

            # ---------------- resident inputs ----------------
            lkm = cp.tile([128, 4, FREE], F32R)
            nc.sync.dma_start(out=lkm[:], in_=lkm_p[:].rearrange("k d f -> d k f"))
            img = cp.tile([128, 4, BPC], F32R)
            nc.sync.dma_start(out=img[:], in_=img_p[:].rearrange("k d f -> d k f"))
            w5 = cp.tile([128, 4, BPC + 1], F32R)
            nc.sync.dma_start(out=w5[:], in_=w5_p[:].rearrange("k d f -> d k f"))
            mtk = cp.tile([128, 4, NC], F32R)
            nc.sync.dma_start(out=mtk[:], in_=mtk_p[:].rearrange("k d f -> d k f"))

            contribs_d = dp.tile([BPC, GP], F32)

            # ---------------- phase W: w_sel -----------------
            ps_w = pp.tile([BPC + 1, FREE], F32, bufs=1)
            for k in range(4):
                nc.tensor.matmul(ps_w[:, 0:512], w5[:, k, :], lkm[:, k, 0:512],
                                 start=(k == 0), stop=(k == 3))
                nc.tensor.matmul(ps_w[:, 512:FREE], w5[:, k, :], lkm[:, k, 512:FREE],
                                 start=(k == 0), stop=(k == 3))
            ws_all = cp.tile([BPC + 1, FREE], F32)
            nc.scalar.copy(out=ws_all[:], in_=ps_w[:])

            s04 = cp.tile([BPC, N], F32)
            nc.sync.dma_start(out=s04[:], in_=ws_all[BPC:BPC + 1, :])
            w4 = cp.tile([BPC, N], F32)
            for b in range(BPC):
                nc.sync.dma_start(out=w4[b:b + 1, :],
                                  in_=ws_all[b:b + 1, b * N:(b + 1) * N])

            s0keep = cp.tile([BPC, N], F32)
            nc.scalar.copy(out=s0keep[:], in_=s04[:])
            m56 = cp.tile([BPC, K56], F32)
            for r in range(7):
                nc.vector.max(out=m56[:, r * 8:(r + 1) * 8], in_=s04[:])
                if r < 6:
                    nc.vector.match_replace(out=s04[:],
                                            in_to_replace=m56[:, r * 8:(r + 1) * 8],
                                            in_values=s04[:], imm_value=-1e30)

            # gather w at the top-k positions: onehot[k,m] = (s0[m] == m56[k])
            eq3 = cp.tile([BPC, K56 * N], F32)
            w4b = w4[:].rearrange("p (o m) -> p o m", o=1).to_broadcast([BPC, K56, N])
            s0b = s0keep[:].rearrange("p (o m) -> p o m", o=1).to_broadcast([BPC, K56, N])
            m56b = m56[:].rearrange("p (k o) -> p k o", o=1).to_broadcast([BPC, K56, N])
            nc.vector.tensor_tensor(out=eq3[:].rearrange("p (a m) -> p a m", a=K56),
                                    in0=m56b, in1=s0b, op=OP.is_equal)
            nc.vector.tensor_tensor(out=eq3[:].rearrange("p (a m) -> p a m", a=K56),
                                    in0=eq3[:].rearrange("p (a m) -> p a m", a=K56),
                                    in1=w4b, op=OP.mult)
            wg = cp.tile([BPC, K56], F32)
            nc.vector.reduce_sum(out=wg[:], in_=eq3[:].rearrange("p (a m) -> p a m", a=K56),
                                 axis=AX)

            wselp = cp.tile([BPC, K56], F32)
            nc.vector.memset(wselp[:], 0.0)
            wsum = cp.tile([BPC, 1], F32)
            nc.scalar.activation(out=wselp[:, 0:KTOP], in_=wg[:, 0:KTOP],
                                 func=ACT.Exp, accum_out=wsum[:])
            wrec = cp.tile([BPC, 1], F32)
            nc.vector.reciprocal(out=wrec[:], in_=wsum[:])
            nc.vector.tensor_scalar_mul(wselp[:, 0:KTOP], wselp[:, 0:KTOP], wrec[:])

            wflat = cp.tile([1, BPC * K56], F32)
            nc.sync.dma_start(out=wflat[:], in_=wselp[:])
            ones = cp.tile([1, 128], F32)
            nc.vector.memset(ones[:], 1.0)
            bc_ps = pp.tile([128, BPC * K56], F32, bufs=1)
            nc.tensor.matmul(bc_ps[:], ones[:], wflat[:], start=True, stop=True)
            wrep = cp.tile([128, BPC * K56], F32)
            nc.scalar.copy(out=wrep[:], in_=bc_ps[:])

            # ------------- phase V state (filled inside main loop) -------------
            mtcbs, vlogs, vexps = [], [], []
            for cb in range(4):
                mtcbs.append(cp.tile([128, D], F32, tag=f"mtc{cb}", name=f"mtcb{cb}"))
                vlogs.append(cp.tile([128, ND], F32, tag=f"vlog{cb}", name=f"vlog{cb}"))
                vexps.append(cp.tile([128, ND], F32, tag=f"vexp{cb}", name=f"vexp{cb}"))
            for cb in range(4):
                cr = min(128, NC - cb * 128)
                nc.sync.dma_start(out=mtcbs[cb][:cr, :],
                                  in_=mtc_p[cb * 128:cb * 128 + cr, :])

            def v_item(j):
                cb, n = j // ND, j % ND
                cr = min(128, NC - cb * 128)
                acn_t = scr.tile([128, D], F32, tag="acn", bufs=3, name=f"acn{j}")
                nc.scalar.dma_start(out=acn_t[:cr, :],
                                    in_=acn_p[cb * 128:cb * 128 + cr, n, :])
                vj = scr.tile([128, D], F32, tag="vjunk", bufs=3, name=f"vj{j}")
                nc.gpsimd.tensor_tensor(out=vj[:cr, :], in0=acn_t[:cr, :],
                                        in1=mtcbs[cb][:cr, :], op=OP.mult)
                vj2 = scr.tile([128, D], F32, tag="vjunk2", bufs=2, name=f"vj2{j}")
                nc.scalar.activation(out=vj2[:cr, :], in_=vj[:cr, :],
                                     func=ACT.Copy,
                                     accum_out=vlogs[cb][:cr, n:n + 1])
                if n == ND - 1:
                    vsum = cp.tile([128, 1], F32, tag=f"vsum{cb}", name=f"vsum{cb}")
                    nc.scalar.activation(out=vexps[cb][:cr, :], in_=vlogs[cb][:cr, :],
                                         func=ACT.Exp, accum_out=vsum[:cr, :])
                    vrec = cp.tile([128, 1], F32, tag=f"vrec{cb}", name=f"vrec{cb}")
                    nc.vector.reciprocal(out=vrec[:cr, :], in_=vsum[:cr, :])
                    nc.vector.tensor_scalar_mul(vexps[cb][:cr, :], vexps[cb][:cr, :],
                                                vrec[:cr, :])

            # ---------------- main loop ----------------------
            LAG = 3
            pending = []

            def flush_tail(t):
                tt, mv = pending.pop(0)
                prod = scr.tile([128, BPC * K56], F32, tag="prod", bufs=3,
                                name=f"prod{tt}")
                nc.gpsimd.tensor_tensor(out=prod[:],
                                        in0=mv[:].rearrange("p a k -> p (a k)"),
                                        in1=wrep[:], op=OP.mult)
                ct = ctp.tile([128, BPC], F32, tag="ct", name=f"ct{tt}")
                nc.vector.reduce_sum(out=ct[:],
                                     in_=prod[:].rearrange("p (a k) -> p a k", a=BPC),
                                     axis=AX)
                nc.sync.dma_start(
                    out=contribs_d[:, tt * 128:(tt + 1) * 128].rearrange("b p -> p b"),
                    in_=ct[:])

            for t in range(NT):
                tkt = tkp.tile([128, 4, 128], F32R)
                nc.sync.dma_start(out=tkt[:], in_=tkc_p[t, :, :])
                st = pp.tile([128, FREE], F32, tag="st", bufs=2)
                for k in range(4):
                    nc.tensor.matmul(st[:, 0:512], tkt[:, k, :], lkm[:, k, 0:512],
                                     start=(k == 0), stop=(k == 3))
                    nc.tensor.matmul(st[:, 512:FREE], tkt[:, k, :], lkm[:, k, 512:FREE],
                                     start=(k == 0), stop=(k == 3))
                sim = simp.tile([128, FREE], F32, tag="sim")
                nc.scalar.copy(out=sim[:], in_=st[:])

                mv3 = mvp.tile([128, BPC, K56], F32, tag="maxv", name=f"mv_{t}")
                for r in range(7):
                    for b in range(BPC):
                        nc.vector.max(out=mv3[:, b, r * 8:(r + 1) * 8],
                                      in_=sim[:, b * N:(b + 1) * N])
                    if r < 6:
                        for b in range(BPC):
                            nc.vector.match_replace(
                                out=sim[:, b * N:(b + 1) * N],
                                in_to_replace=mv3[:, b, r * 8:(r + 1) * 8],
                                in_values=sim[:, b * N:(b + 1) * N],
                                imm_value=-1e30)

                pending.append((t, mv3))
                if len(pending) > LAG:
                    flush_tail(t)

                # interleave v work across the first tiles
                for j in range(t * NV // NT, (t + 1) * NV // NT):
                    v_item(j)

            while pending:
                flush_tail(NT)

            # ---------------- finale -------------------------
            for cb in range(4):
                cr = min(128, NC - cb * 128)
                rb = cp.tile([128, BPC * ND], F32, tag=f"rb{cb}", name=f"rb{cb}")
                nc.sync.dma_start(
                    out=rb[:cr, :],
                    in_=contribs_d[:, cb * CBLK:cb * CBLK + cr * ND]
                    .rearrange("b (p n) -> p b n", n=ND))
                vb = vexps[cb][:cr, :].rearrange("p (o n) -> p o n", o=1) \
                    .to_broadcast([cr, BPC, ND])
                nc.vector.tensor_tensor(out=rb[:cr, :].rearrange("p (b n) -> p b n", n=ND),
                                        in0=rb[:cr, :].rearrange("p (b n) -> p b n", n=ND),
                                        in1=vb, op=OP.mult)
                bias4 = cp.tile([128, BPC], F32, tag=f"bias{cb}", name=f"bias{cb}")
                nc.vector.reduce_sum(out=bias4[:cr, :],
                                     in_=rb[:cr, :].rearrange("p (b n) -> p b n", n=ND),
                                     axis=AX)
                pb = pp.tile([128, BPC], F32, tag="pb", bufs=1)
                for k in range(4):
                    nc.tensor.matmul(pb[:cr, :], mtk[:, k, cb * 128:cb * 128 + cr],
                                     img[:, k, :], start=(k == 0), stop=(k == 3))
                o4 = cp.tile([128, BPC], F32, tag=f"o4{cb}", name=f"o4{cb}")
                nc.vector.tensor_tensor(out=o4[:cr, :], in0=bias4[:cr, :],
                                        in1=pb[:cr, :], op=OP.add)
                nc.sync.dma_start(
                    out=out_p[:, cb * 128:cb * 128 + cr].rearrange("b c -> c b"),
                    in_=o4[:cr, :])

    nc.finalize()
    return nc


def kernel(image_features, local_image_features, all_text_features,
           mean_text_features, topk):
    global LAST_EXEC_NS, _PROGRAM
    assert int(topk) == KTOP
    _install_ntff_hook()
    from concourse.bass_utils import run_bass_kernel_spmd

    imgf = np.ascontiguousarray(np.asarray(image_features, dtype=np.float32))
    locf = np.ascontiguousarray(np.asarray(local_image_features, dtype=np.float32))
    txtf = np.ascontiguousarray(np.asarray(all_text_features, dtype=np.float32))
    mtf = np.ascontiguousarray(np.asarray(mean_text_features, dtype=np.float32))

    # text cols c-major: col j = c*51+n  ->  all_text[n,c,:]
    tp = np.zeros((D, GP), dtype=np.float32)
    tp[:, :G] = txtf.transpose(2, 1, 0).reshape(D, G)
    # tile-major: tkc[t, dp, k, f] = tp[k*128+dp, t*128+f] -> contiguous 2KB/partition
    tkc = np.ascontiguousarray(
        tp.reshape(4, 128, NT, 128).transpose(2, 1, 0, 3)).reshape(NT, 128, 512)
    mtk = mtf.T.reshape(4, 128, NC).copy()
    acn = txtf.transpose(1, 0, 2).copy()           # [c, n, d]
    t00 = txtf[0, 0, :]                            # class 0, descriptor 0

    if _PROGRAM is None:
        _PROGRAM = _build_program()
    nc = _PROGRAM

    in_maps = []
    for ci in range(CORES):
        sl = slice(ci * BPC, (ci + 1) * BPC)
        li = locf[sl]                              # [4, 197, 512]
        lkm = li.transpose(2, 0, 1).reshape(D, FREE).reshape(4, 128, FREE).copy()
        im = imgf[sl].T.reshape(4, 128, BPC).copy()
        w5 = np.concatenate([imgf[sl].T, t00[:, None]], axis=1) \
            .reshape(4, 128, BPC + 1).copy()
        in_maps.append({
            "tkc": tkc, "lkm": lkm, "img": im, "w5": w5,
            "mtk": mtk, "mtc": mtf, "acn": acn,
        })

    res = run_bass_kernel_spmd(nc, in_maps, core_ids=list(range(CORES)))
    LAST_EXEC_NS = res.exec_time_ns
    out = np.concatenate([res.results[ci]["out"] for ci in range(CORES)], axis=0)
    return out.astype(np.float32)
